# revision 9
# baseline (speedup 1.0000x reference)
"""Multi-head causal attention (B=4, S=2048, D=1024, H=16) on 8 TRN2 NeuronCores.

Sharding: core c handles batch b = c//2 and heads h in [8*(c%2), 8*(c%2)+8)
(tensor parallel on heads x data parallel on batch). Each core computes its
partial output projection ctx_h @ Wo[:, h-cols].T; the host sums the two
partials per batch and adds bo.

Per-core device kernel (all matmuls bf16, fp32 PSUM accumulation):
  - projections produce qT/kT in [head_dim, S] layout and V in [S, head_dim]
    layout directly (scores are computed transposed: [k, q])
  - softmax: exp on ScalarE (scale=1/8 fused), causal mask = multiply by 0/1
    mask tile, row-sums come for free from a ones-column appended to V
    (M=65 PV matmul), normalization via DVE reciprocal + gpsimd
    partition_broadcast
  - 2-head row-packing (K=64) for the score matmuls
"""

import numpy as np
import ml_dtypes

import concourse.bacc as bacc
import concourse.mybir as mybir
import concourse.tile as tile
from concourse.bass_utils import run_bass_kernel_spmd

BF16 = mybir.dt.bfloat16
F32 = mybir.dt.float32

# problem constants
B, S, D, H = 4, 2048, 1024, 16
HD = 64          # head dim
HPC = 8          # heads per core
DH = HPC * HD    # 512 per-core head dims
N_CORES = 8

P = 128          # partitions
QB = 512         # q block (matmul free dim)


def build_core_kernel(s=S, d=D, hpc=HPC):
    """Build the per-core Bass kernel. Parameterized for small-scale testing."""
    dh = hpc * HD
    n_dt = d // P          # D k-tiles (contraction tiles for projections)
    n_mt = dh // P         # dh tiles (also head-pairs)
    n_st = s // P          # sequence tiles of 128
    n_qb = s // QB         # q blocks of 512
    kt_per_qb = QB // P    # 4 k-tiles per q block

    nc = bacc.Bacc("TRN2", target_bir_lowering=False, debug=False,
                   num_devices=1)

    xT = nc.dram_tensor("xT", [d, s], BF16, kind="ExternalInput").ap()
    wqT = nc.dram_tensor("wqT", [d, dh], BF16, kind="ExternalInput").ap()
    wkT = nc.dram_tensor("wkT", [d, dh], BF16, kind="ExternalInput").ap()
    wvT = nc.dram_tensor("wvT", [d, dh], BF16, kind="ExternalInput").ap()
    woT = nc.dram_tensor("woT", [dh, d], BF16, kind="ExternalInput").ap()
    maskin = nc.dram_tensor("maskin", [P, 896], BF16, kind="ExternalInput").ap()
    out = nc.dram_tensor("out", [s, d], F32, kind="ExternalOutput").ap()
    out_t = out.rearrange("(t p) d2 -> p t d2", p=P)

    with tile.TileContext(nc) as tc:
        with (
            tc.tile_pool(name="wts", bufs=1) as wts,
            tc.tile_pool(name="xt", bufs=1) as xtp,
            tc.tile_pool(name="qkv", bufs=1) as qkv,
            tc.tile_pool(name="attn", bufs=6) as attn,
            tc.tile_pool(name="norm", bufs=3) as norm,
            tc.tile_pool(name="outp", bufs=3) as outp,
            tc.tile_pool(name="pproj", bufs=2, space="PSUM") as pproj,
            tc.tile_pool(name="pscore", bufs=2, space="PSUM") as pscore,
            tc.tile_pool(name="ppv", bufs=2, space="PSUM") as ppv,
        ):
            # ---- static SBUF tensors ----
            wq_sb = wts.tile([P, n_dt, dh], BF16, tag="wq")
            wk_sb = wts.tile([P, n_dt, dh], BF16, tag="wk")
            wv_sb = wts.tile([P, n_dt, dh], BF16, tag="wv")
            wo_sb = wts.tile([P, n_mt, d], BF16, tag="wo")
            mask_sb = wts.tile([P, 896], BF16, tag="mask")
            xt_sb = xtp.tile([P, n_dt, s], BF16, tag="xt")
            q_sb = qkv.tile([P, n_mt, s], BF16, tag="q")
            k_sb = qkv.tile([P, n_mt, s], BF16, tag="k")
            # V with a ones column appended per head: [s-tile][head][65]
            v_sb = qkv.tile([P, n_st, hpc, HD + 1], BF16, tag="v")
            ctx_sb = qkv.tile([P, n_mt, s], BF16, tag="ctx")

            nc.sync.dma_start(wq_sb[:], wqT.rearrange("(o p) m -> p o m", p=P))
            nc.sync.dma_start(wk_sb[:], wkT.rearrange("(o p) m -> p o m", p=P))
            nc.sync.dma_start(xt_sb[:], xT.rearrange("(o p) n -> p o n", p=P))
            nc.sync.dma_start(wv_sb[:], wvT.rearrange("(o p) m -> p o m", p=P))
            nc.sync.dma_start(wo_sb[:], woT.rearrange("(o p) m -> p o m", p=P))
            nc.sync.dma_start(mask_sb[:], maskin[:])
            nc.vector.memset(v_sb[:, :, :, HD], 1.0)

            # ---- emission helpers ----
            def proj_qk(m):
                """qT and kT for dh-tile m (head pair m)."""
                for w_sb, dst in ((wq_sb, q_sb), (wk_sb, k_sb)):
                    for n in range(s // QB):
                        ps = pproj.tile([P, QB], F32, tag="proj")
                        for kd in range(n_dt):
                            nc.tensor.matmul(
                                ps[:],
                                w_sb[:, kd, m * P:(m + 1) * P],
                                xt_sb[:, kd, n * QB:(n + 1) * QB],
                                start=(kd == 0), stop=(kd == n_dt - 1))
                        nc.vector.tensor_copy(
                            dst[:, m, n * QB:(n + 1) * QB], ps[:])

            def proj_v(s_lo, s_hi):
                """V for sequence tiles [s_lo, s_hi)."""
                for st in range(s_lo, s_hi):
                    ps = pproj.tile([P, hpc, HD], F32, tag="proj")
                    for kd in range(n_dt):
                        nc.tensor.matmul(
                            ps[:],
                            xt_sb[:, kd, st * P:(st + 1) * P],
                            wv_sb[:, kd, :],
                            start=(kd == 0), stop=(kd == n_dt - 1))
                    nc.vector.tensor_copy(v_sb[:, st, :, :HD], ps[:])

            def att_block(hp, qb):
                """Attention for head pair (2hp, 2hp+1), q block qb."""
                heads = (2 * hp, 2 * hp + 1)
                n_kt = kt_per_qb * (qb + 1)      # k-tiles in causal range
                pv = [ppv.tile([P, QB], F32, tag="pv", name=f"pv{hi}")
                      for hi in range(2)]
                qs = slice(qb * QB, (qb + 1) * QB)
                for c0 in range(0, n_kt, 2):     # chunks of 2 k-tiles
                    nkt_c = min(2, n_kt - c0)
                    sc = [pscore.tile([P, 2 * QB], F32, tag="sc", name=f"sc{hi}")
                          for hi in range(2)]
                    for j in range(nkt_c):       # scores (2-head row-packed)
                        kt = c0 + j
                        for hi, h in enumerate(heads):
                            pr = slice((h % 2) * 64, (h % 2) * 64 + 64)
                            nc.tensor.matmul(
                                sc[hi][:, j * QB:(j + 1) * QB],
                                k_sb[pr, hp, kt * P:(kt + 1) * P],
                                q_sb[pr, hp, qs],
                                start=True, stop=True)
                    ex = [attn.tile([P, 2 * QB], BF16, tag="ex", name=f"ex{hi}")
                          for hi in range(2)]
                    for hi in range(2):
                        nc.scalar.activation(
                            ex[hi][:, :nkt_c * QB], sc[hi][:, :nkt_c * QB],
                            mybir.ActivationFunctionType.Exp, scale=0.125)
                    for j in range(nkt_c):       # causal mask on diag tiles
                        kt = c0 + j
                        delta = (kt - kt_per_qb * qb) * P
                        if delta >= 0:           # diagonal-crossing tile
                            msl = mask_sb[:, 384 - delta:896 - delta]
                            for hi in range(2):
                                sl = ex[hi][:, j * QB:(j + 1) * QB]
                                nc.vector.tensor_tensor(
                                    sl, sl, msl, mybir.AluOpType.mult)
                    for j in range(nkt_c):       # PV with ones column (M=65)
                        kt = c0 + j
                        for hi, h in enumerate(heads):
                            nc.tensor.matmul(
                                pv[hi][:HD + 1, :],
                                v_sb[:, kt, h, :],
                                ex[hi][:, j * QB:(j + 1) * QB],
                                start=(kt == 0), stop=(kt == n_kt - 1))
                # normalize: ctxT = pv[:64] * (1 / rowsum)
                for hi, h in enumerate(heads):
                    rec = norm.tile([1, QB], F32, tag="rec")
                    nc.vector.reciprocal(rec[:], pv[hi][HD:HD + 1, :])
                    bc = norm.tile([64, QB], F32, tag="bc")
                    nc.gpsimd.partition_broadcast(bc[:], rec[:])
                    pr = slice((h % 2) * 64, (h % 2) * 64 + 64)
                    nc.vector.tensor_tensor(
                        ctx_sb[pr, hp, qs], pv[hi][:HD, :], bc[:],
                        mybir.AluOpType.mult)

            def out_proj(s_lo, s_hi):
                """Output projection for sequence tiles [s_lo, s_hi)."""
                ob = min(QB, d)
                for st in range(s_lo, s_hi):
                    o_sb = outp.tile([P, d], F32, tag="o")
                    for n in range(d // ob):
                        ps = pproj.tile([P, QB], F32, tag="proj")
                        for mt in range(n_mt):
                            nc.tensor.matmul(
                                ps[:, :ob],
                                ctx_sb[:, mt, st * P:(st + 1) * P],
                                wo_sb[:, mt, n * ob:(n + 1) * ob],
                                start=(mt == 0), stop=(mt == n_mt - 1))
                        nc.vector.tensor_copy(
                            o_sb[:, n * ob:(n + 1) * ob], ps[:, :ob])
                    nc.sync.dma_start(out_t[:, st, :], o_sb[:])

            # ---- emission schedule (hand-interleaved for ACT/PE overlap) ----
            if n_mt == 4 and n_qb == 4 and n_st == 16:
                proj_qk(0)
                att_block(0, 0)
                proj_qk(1)
                att_block(1, 0)
                proj_v(0, 4)
                att_block(0, 1)
                proj_qk(2)
                att_block(2, 0)
                proj_v(4, 8)
                att_block(1, 1)
                proj_qk(3)
                att_block(3, 0)
                proj_v(8, 12)
                att_block(2, 1)
                att_block(0, 2)
                proj_v(12, 16)
                att_block(3, 1)
                att_block(1, 2)
                att_block(0, 3)
                att_block(2, 2)
                att_block(1, 3)
                out_proj(0, 2)
                att_block(2, 3)
                out_proj(2, 6)
                att_block(3, 2)
                out_proj(6, 8)
                att_block(3, 3)
                out_proj(8, 16)
            else:  # generic order for small test configs
                for m in range(n_mt):
                    proj_qk(m)
                proj_v(0, n_st)
                for hp in range(n_mt):
                    for qb in range(n_qb):
                        att_block(hp, qb)
                out_proj(0, n_st)

    nc.compile()
    return nc


def _causal_ext_mask():
    """[128, 896] bf16: m[k, j] = 1.0 if j - 384 >= k else 0.0."""
    j = np.arange(896)[None, :]
    k = np.arange(P)[:, None]
    return (j - 384 >= k).astype(ml_dtypes.bfloat16)


_NC_CACHE = {}
_RUN_KW = {}


def profile_once(inputs):
    """Run once with tracing and return slowest-core exec time in ns."""
    global _RUN_KW
    _RUN_KW = {"trace": True, "trace_cores": [0]}
    try:
        kernel(**inputs)
    finally:
        _RUN_KW = {}
    res = _NC_CACHE.get("last_results")
    return None if res is None else res.exec_time_ns


def measure_hw_ns(in_maps_or_inputs, iters=48, nc=None, n_cores=None):
    """Amortized per-execution time of the NEFF via async PJRT dispatch.

    Keeps inputs device-resident and queues `iters` executions without
    blocking, so the axon tunnel latency pipelines away; returns ns/iter.
    """
    import time as _time
    import jax
    import jax.numpy as jnp  # noqa: F401
    from jax.sharding import Mesh, PartitionSpec
    from jax.experimental.shard_map import shard_map
    from concourse import bass2jax
    import concourse.mybir as _mybir

    if isinstance(in_maps_or_inputs, dict):
        in_maps = _make_in_maps(**in_maps_or_inputs)
    else:
        in_maps = in_maps_or_inputs
    if nc is None:
        if "full" not in _NC_CACHE:
            _NC_CACHE["full"] = build_core_kernel()
        nc = _NC_CACHE["full"]
    if n_cores is None:
        n_cores = len(in_maps)

    bass2jax.install_neuronx_cc_hook()
    part_name = nc.partition_id_tensor.name if nc.partition_id_tensor else None
    in_names, out_names, out_avals, zero_outs = [], [], [], []
    for alloc in nc.m.functions[0].allocations:
        if not isinstance(alloc, _mybir.MemoryLocationSet):
            continue
        name = alloc.memorylocations[0].name
        if alloc.kind == "ExternalInput":
            if name != part_name:
                in_names.append(name)
        elif alloc.kind == "ExternalOutput":
            out_names.append(name)
            shape = tuple(alloc.tensor_shape)
            dtype = _mybir.dt.np(alloc.dtype)
            out_avals.append(jax.core.ShapedArray(shape, dtype))
            zero_outs.append(np.zeros(shape, dtype))
    n_params = len(in_names)
    all_names = in_names + out_names

    if part_name is not None:
        all_names = all_names + [part_name]

    def _body(*args):
        operands = list(args)
        if part_name is not None:
            operands.append(bass2jax.partition_id_tensor())
        return tuple(bass2jax._bass_exec_p.bind(
            *operands, out_avals=tuple(out_avals), in_names=tuple(all_names),
            out_names=tuple(out_names), lowering_input_output_aliases=(),
            sim_require_finite=False, sim_require_nnan=False, nc=nc))

    devices = jax.devices()[:n_cores]
    mesh = Mesh(np.asarray(devices), ("core",))
    fn = jax.jit(shard_map(
        _body, mesh=mesh,
        in_specs=(PartitionSpec("core"),) * (n_params + len(out_names)),
        out_specs=(PartitionSpec("core"),) * len(out_names),
        check_rep=False))
    concat = [np.concatenate([np.asarray(in_maps[c][n]) for c in range(n_cores)],
                             axis=0) for n in in_names]
    concat += [np.concatenate([z] * n_cores, axis=0) for z in zero_outs]
    dev_args = [jax.device_put(a) for a in concat]
    outs = fn(*dev_args)
    jax.block_until_ready(outs)
    t0 = _time.perf_counter()
    for _ in range(iters):
        outs = fn(*dev_args)
    jax.block_until_ready(outs)
    return (_time.perf_counter() - t0) / iters * 1e9


def _make_in_maps(x, Wq, Wk, Wv, Wo, bo=None):
    x = np.asarray(x, dtype=np.float32)
    mask = _causal_ext_mask()
    bf = ml_dtypes.bfloat16
    xT = [np.ascontiguousarray(x[b].T).astype(bf) for b in range(B)]
    in_maps = []
    for c in range(N_CORES):
        b, g = c // 2, c % 2
        rows = slice(g * DH, (g + 1) * DH)
        in_maps.append({
            "xT": xT[b],
            "wqT": np.ascontiguousarray(np.asarray(Wq, np.float32)[rows, :].T).astype(bf),
            "wkT": np.ascontiguousarray(np.asarray(Wk, np.float32)[rows, :].T).astype(bf),
            "wvT": np.ascontiguousarray(np.asarray(Wv, np.float32)[rows, :].T).astype(bf),
            "woT": np.ascontiguousarray(np.asarray(Wo, np.float32)[:, rows].T).astype(bf),
            "maskin": mask,
        })
    return in_maps


def kernel(x, Wq, Wk, Wv, Wo, bo):
    bo = np.asarray(bo, dtype=np.float32)

    if "full" not in _NC_CACHE:
        _NC_CACHE["full"] = build_core_kernel()
    nc = _NC_CACHE["full"]

    in_maps = _make_in_maps(x, Wq, Wk, Wv, Wo)

    res = run_bass_kernel_spmd(nc, in_maps, core_ids=list(range(N_CORES)),
                               **_RUN_KW)
    outs = [r["out"] for r in res.results]
    _NC_CACHE["last_results"] = res
    full = np.empty((B, S, D), dtype=np.float32)
    for b in range(B):
        full[b] = outs[2 * b] + outs[2 * b + 1]
    if np.any(bo):
        full += bo[None, None, :]
    return full
